# revision 7
# baseline (speedup 1.0000x reference)
"""Multi-head causal attention (B=4,T=2048,D=1024,H=16) on 8 TRN2 NeuronCores.

Sharding: data-parallel on B (4 batches x 2 core-groups), tensor-parallel on
heads (8 heads per core). Each core computes, for its (b, 8-head) slice:
  - w[8,2048,2048]: softmax attention weights (lower triangle written;
    upper triangle left unwritten -> returned as zeros by the pre-zeroed
    donated output buffer)
  - outT[1024,2048]: transposed partial projection output; host adds the two
    per-batch partials and transposes.

Self-contained: hardcodes all shapes; only imports the environment-provided
concourse/axon stack.
"""

import contextlib
import ctypes
import sys
import types

import numpy as np

import concourse.bass as bass
import concourse.mybir as mybir
import concourse.tile as tile
from concourse import bacc
from concourse.bass_utils import run_bass_kernel_spmd

F32 = mybir.dt.float32
F32R = mybir.dt.float32r
BF16 = mybir.dt.bfloat16
AF = mybir.ActivationFunctionType

B, T, D, H, DH = 4, 2048, 1024, 16, 64
HPC = 8          # heads per core
N_CORES = 8
SCALE = 1.0 / 8.0  # 1/sqrt(DH)
NEG = -1e30

_NC = None  # cached compiled Bass module


# ---------------------------------------------------------------- ntff hook
def _install_ntff_hook():
    """The agent image's antenv lacks axon_hooks; replicate the ctypes NTFF
    profile hook so run_bass_kernel_spmd(trace=True) works."""
    if "antenv.axon_hooks" in sys.modules:
        return
    so_path = "/opt/axon/libaxon_pjrt.so"
    try:
        lib = ctypes.CDLL(so_path)
        assert hasattr(lib, "axon_start_nrt_profile")
        lib.axon_start_nrt_profile.argtypes = [ctypes.POINTER(ctypes.c_int64), ctypes.c_size_t]
        lib.axon_start_nrt_profile.restype = ctypes.c_int64
        lib.axon_stop_nrt_profile.argtypes = [ctypes.c_char_p]
        lib.axon_stop_nrt_profile.restype = ctypes.c_int64

        @contextlib.contextmanager
        def _hook(output_dir, device_ids):
            import jax
            jax.devices()
            if device_ids:
                ids = (ctypes.c_int64 * len(device_ids))(*device_ids)
                rc = lib.axon_start_nrt_profile(ids, len(device_ids))
            else:
                rc = lib.axon_start_nrt_profile(None, 0)
            if rc != 0:
                raise RuntimeError(f"axon_start_nrt_profile rc={rc}")
            try:
                yield
            finally:
                n = lib.axon_stop_nrt_profile(str(output_dir).encode())
                print(f"ntff profile: {n} file(s) -> {output_dir}", file=sys.stderr)
    except Exception:
        _hook = None
    mod = types.ModuleType("antenv.axon_hooks")
    mod.get_axon_ntff_profile_hook = lambda: _hook
    mod.set_axon_ntff_profile_hook = lambda h: None
    sys.modules["antenv.axon_hooks"] = mod


# ---------------------------------------------------------------- builder
def _build():
    nc = bacc.Bacc("TRN2", target_bir_lowering=False, debug=False,
                   num_devices=N_CORES)
    xT_d = nc.dram_tensor("xT", [D, T], F32R, kind="ExternalInput").ap()
    wqk_d = nc.dram_tensor("wqk", [D, 2 * HPC * DH], F32R, kind="ExternalInput").ap()
    wv_d = nc.dram_tensor("wv", [D, HPC * DH], F32R, kind="ExternalInput").ap()
    wp_d = nc.dram_tensor("wp", [HPC * DH, D], F32R, kind="ExternalInput").ap()
    w_out = nc.dram_tensor("w_out", [HPC, T, T], F32, kind="ExternalOutput").ap()
    outT_d = nc.dram_tensor("outT", [D, T], F32, kind="ExternalOutput").ap()

    NQK = 2 * HPC * DH   # 1024 (q rows | k rows)
    NV = HPC * DH        # 512
    NT = T // 128        # 16 key tiles
    NCH = T // 512       # 4 query chunks

    with tile.TileContext(nc) as tc:
        with contextlib.ExitStack() as ctx:
            consts = ctx.enter_context(tc.tile_pool(name="consts", bufs=1))
            persist = ctx.enter_context(tc.tile_pool(name="persist", bufs=1))

            # constants: bf16 identity, transposed additive causal mask
            # maskN[x(query), y(key)] = 0 if x >= y else NEG
            maskN = consts.tile([128, 128], F32)
            nc.gpsimd.memset(maskN[:], 0.0)
            nc.gpsimd.affine_select(out=maskN[:], in_=maskN[:],
                                    compare_op=mybir.AluOpType.is_ge,
                                    fill=NEG, base=0, pattern=[[-1, 128]],
                                    channel_multiplier=1)
            # maskT[x(key), y(query)] = 0 if y >= x else NEG
            maskT = consts.tile([128, 128], F32)
            nc.gpsimd.memset(maskT[:], 0.0)
            nc.gpsimd.affine_select(out=maskT[:], in_=maskT[:],
                                    compare_op=mybir.AluOpType.is_ge,
                                    fill=NEG, base=0, pattern=[[1, 128]],
                                    channel_multiplier=-1)

            # persistent tensors
            qkT = [persist.tile([128, T], F32R, tag=f"qkT{g}", name=f"qkT{g}")
                   for g in range(8)]
            v_aug = [persist.tile([128, HPC, DH + 1], BF16, tag=f"vaug{k}",
                                    name=f"vaug{k}")
                     for k in range(NT)]

            # ---------------- phase A: QKV projections ----------------
            with contextlib.ExitStack() as pa:
                wpool = pa.enter_context(tc.tile_pool(name="wpool", bufs=1))
                xpool = pa.enter_context(tc.tile_pool(name="xpool", bufs=2))
                qk_ps = pa.enter_context(tc.tile_pool(name="qk_ps", bufs=2, space="PSUM"))
                v_ps = pa.enter_context(tc.tile_pool(name="v_ps", bufs=2, space="PSUM"))

                wqk_sb = wpool.tile([128, 8, NQK], F32R)
                nc.sync.dma_start(wqk_sb[:], wqk_d.rearrange("(c p) d -> p c d", p=128))
                wv_sb = wpool.tile([128, 8, NV], F32R)
                nc.sync.dma_start(wv_sb[:], wv_d.rearrange("(c p) d -> p c d", p=128))
                ones8 = wpool.tile([128, HPC], F32)
                nc.vector.memset(ones8[:], 1.0)

                for t4 in range(NCH):
                    xt = xpool.tile([128, 8, 512], F32R, tag="xt")
                    nc.sync.dma_start(
                        xt[:], xT_d[:, 512 * t4:512 * (t4 + 1)]
                        .rearrange("(c p) t -> p c t", p=128))
                    for dt in range(8):
                        P = qk_ps.tile([128, 512], F32, tag="qk")
                        for cc in range(8):
                            nc.tensor.matmul(P[:], wqk_sb[:, cc, 128 * dt:128 * (dt + 1)],
                                             xt[:, cc, :], start=(cc == 0), stop=(cc == 7))
                        nc.vector.tensor_copy(qkT[dt][:, 512 * t4:512 * (t4 + 1)], P[:])
                    for tt in range(4):
                        kt = 4 * t4 + tt
                        P = v_ps.tile([128, 512], F32, tag="v")
                        for cc in range(8):
                            nc.tensor.matmul(P[:], xt[:, cc, 128 * tt:128 * (tt + 1)],
                                             wv_sb[:, cc, :], start=(cc == 0), stop=(cc == 7))
                        nc.vector.tensor_copy(
                            v_aug[kt][:, :, 0:DH],
                            P[:].rearrange("p (h d) -> p h d", h=HPC))
                        nc.vector.tensor_copy(v_aug[kt][:, :, DH], ones8[:])

            # ---------------- phase B: attention + projection ----------------
            with contextlib.ExitStack() as pb:
                bpool = pb.enter_context(tc.tile_pool(name="bpool", bufs=1))
                ctxT = [bpool.tile([128, T], F32R, tag=f"ctxT{g}", name=f"ctxT{g}")
                        for g in range(4)]
                wp_sb = bpool.tile([128, 4, D], F32R)
                nc.sync.dma_start(wp_sb[:], wp_d.rearrange("(g p) e -> p g e", p=128))

                with contextlib.ExitStack() as pc:
                    et_pool = pc.enter_context(tc.tile_pool(name="et", bufs=6))
                    small = pc.enter_context(tc.tile_pool(name="small", bufs=3))
                    wstage = pc.enter_context(tc.tile_pool(name="wstage", bufs=2))
                    sT_ps = pc.enter_context(tc.tile_pool(name="sT_ps", bufs=2, space="PSUM"))
                    ctx_ps = pc.enter_context(tc.tile_pool(name="ctx_ps", bufs=2, space="PSUM"))
                    en_ps = pc.enter_context(tc.tile_pool(name="en_ps", bufs=2, space="PSUM"))

                    for g in range(4):  # head pairs (base 0 / 64)
                        kTh = qkT[4 + g]
                        qTh = qkT[g]
                        rzch = {}
                        nlzh = {}
                        for bi, base in enumerate((0, 64)):
                            hl = 2 * g + bi
                            rzch[base] = small.tile([128, 16], F32, tag=f"rzch{bi}",
                                                    name=f"rzch_{g}_{bi}")
                        for c in range(NCH):
                            q0 = 512 * c
                            jmax = 4 * c + 3
                            CPs = {}
                            for base in (0, 64):
                                CPs[base] = ctx_ps.tile([DH + 1, 512], F32, tag="ctx",
                                                        name=f"cp_{g}_{c}_{base}")
                            for j in range(jmax + 1):
                                qs = max(q0, 128 * j)
                                N = 512 * (c + 1) - qs
                                d0 = qs - q0
                                Ss = {}
                                for base in (0, 64):  # adjacent -> PE row-group overlap
                                    S = sT_ps.tile([128, 512], F32, tag="sT",
                                                   name=f"sT_{g}_{c}_{j}_{base}")
                                    nc.tensor.matmul(
                                        S[:, :N],
                                        kTh[base:base + 64, 128 * j:128 * (j + 1)],
                                        qTh[base:base + 64, qs:qs + N],
                                        start=True, stop=True, tile_position=(base, 0))
                                    Ss[base] = S
                                for bi, base in enumerate((0, 64)):
                                    hl = 2 * g + bi
                                    S = Ss[base]
                                    if 128 * j >= q0:  # diagonal block at cols 0:128
                                        nc.vector.tensor_add(S[:, 0:128], S[:, 0:128], maskT[:])
                                    ET = et_pool.tile([128, 512], BF16, tag="et",
                                                      name=f"et_{g}_{c}_{j}_{base}")
                                    nc.scalar.activation(ET[:, :N], S[:, :N], AF.Exp, scale=SCALE)
                                    nc.tensor.matmul(CPs[base][:, d0:d0 + N],
                                                     v_aug[j][:, hl, :], ET[:, :N],
                                                     start=(j == 0), stop=(j == jmax))
                            for bi, base in enumerate((0, 64)):
                                hl = 2 * g + bi
                                CP = CPs[base]
                                zrow = small.tile([1, 512], F32, tag=f"zrow{bi}",
                                                  name=f"zrow_{g}_{c}_{bi}")
                                nc.vector.tensor_copy(zrow[:], CP[DH:DH + 1, :])
                                zcol = small.tile([128, 4], F32, tag=f"zcol{bi}",
                                                  name=f"zcol_{g}_{c}_{bi}")
                                for f in range(4):
                                    nc.sync.dma_start(zcol[:, f:f + 1],
                                                      zrow[:, 128 * f:128 * (f + 1)])
                                nc.vector.reciprocal(rzch[base][:, 4 * c:4 * c + 4], zcol[:])
                                rzrow = small.tile([1, 512], F32, tag=f"rzrow{bi}",
                                                   name=f"rzrow_{g}_{c}_{bi}")
                                for f in range(4):
                                    nc.sync.dma_start(rzrow[:, 128 * f:128 * (f + 1)],
                                                      rzch[base][:, 4 * c + f:4 * c + f + 1])
                                bc = small.tile([64, 512], F32, tag=f"bc{bi}",
                                                name=f"bc_{g}_{c}_{bi}")
                                nc.gpsimd.partition_broadcast(bc[:], rzrow[:])
                                nc.vector.tensor_mul(ctxT[g][base:base + 64, q0:q0 + 512],
                                                     CP[0:DH, :], bc[:])
                        # batched -lnZ per head (one Ln each -> fewer table reloads)
                        for bi, base in enumerate((0, 64)):
                            nlzh[base] = small.tile([128, 16], F32, tag=f"nlz{bi}",
                                                    name=f"nlz_{g}_{bi}")
                            nc.scalar.activation(nlzh[base][:], rzch[base][:], AF.Ln)
                        # natural-orientation w: paired matmuls, w = exp(s/8 - lnZ)
                        for i in range(16):
                            nk = 128 * (i + 1)
                            Ws = {}
                            for bi, base in enumerate((0, 64)):
                                Ws[base] = wstage.tile([128, T], F32, tag=f"w{bi}",
                                                       name=f"wst_{g}_{i}_{bi}")
                            for kb in range(0, nk, 1024):
                                width = min(1024, nk - kb)
                                EPs = {}
                                for base in (0, 64):
                                    EPs[base] = en_ps.tile([128, 1024], F32, tag="en",
                                                           name=f"en_{g}_{i}_{kb}_{base}")
                                for s0 in range(0, width, 512):
                                    sw = min(512, width - s0)
                                    for base in (0, 64):  # adjacent pair
                                        nc.tensor.matmul(
                                            EPs[base][:, s0:s0 + sw],
                                            qTh[base:base + 64, 128 * i:128 * (i + 1)],
                                            kTh[base:base + 64, kb + s0:kb + s0 + sw],
                                            start=True, stop=True,
                                            tile_position=(base, 0))
                                for bi, base in enumerate((0, 64)):
                                    hl = 2 * g + bi
                                    EP = EPs[base]
                                    if kb + width == nk:  # diag block: last 128 cols
                                        nc.vector.tensor_add(
                                            EP[:, width - 128:width],
                                            EP[:, width - 128:width], maskN[:])
                                    nc.scalar.activation(
                                        Ws[base][:, kb:kb + width], EP[:, :width],
                                        AF.Exp, scale=SCALE, bias=nlzh[base][:, i:i + 1])
                            for bi, base in enumerate((0, 64)):
                                hl = 2 * g + bi
                                nc.sync.dma_start(
                                    w_out[hl, 128 * i:128 * (i + 1), 0:nk],
                                    Ws[base][:, 0:nk])

                # ---------------- projection ----------------
                with contextlib.ExitStack() as pd:
                    opool = pd.enter_context(tc.tile_pool(name="opool", bufs=3))
                    o_ps = pd.enter_context(tc.tile_pool(name="o_ps", bufs=3, space="PSUM"))
                    for et8 in range(8):
                        for tn in range(NCH):
                            P = o_ps.tile([128, 512], F32, tag="o")
                            for g in range(4):
                                nc.tensor.matmul(P[:],
                                                 wp_sb[:, g, 128 * et8:128 * (et8 + 1)],
                                                 ctxT[g][:, 512 * tn:512 * (tn + 1)],
                                                 start=(g == 0), stop=(g == 3))
                            St = opool.tile([128, 512], F32, tag="st")
                            nc.vector.tensor_copy(St[:], P[:])
                            nc.sync.dma_start(
                                outT_d[128 * et8:128 * (et8 + 1), 512 * tn:512 * (tn + 1)],
                                St[:])
    nc.compile()
    return nc


def _get_nc():
    global _NC
    if _NC is None:
        _NC = _build()
    return _NC


def _prep_in_maps(x, w_qkv, w_proj):
    in_maps = []
    for core in range(N_CORES):
        b = core // 2
        hs = (core % 2) * HPC
        r0 = hs * DH
        wq = w_qkv[r0:r0 + 512]
        wk = w_qkv[D + r0:D + r0 + 512]
        wv = w_qkv[2 * D + r0:2 * D + r0 + 512]
        in_maps.append({
            "xT": np.ascontiguousarray(x[b].T),
            "wqk": np.ascontiguousarray(np.concatenate([wq, wk], axis=0).T),
            "wv": np.ascontiguousarray(wv.T),
            "wp": np.ascontiguousarray(w_proj[:, r0:r0 + 512].T),
        })
    return in_maps


def _assemble(results):
    w = np.empty((B, H, T, T), dtype=np.float32)
    out = np.empty((B, T, D), dtype=np.float32)
    for b in range(B):
        r0, r1 = results[2 * b], results[2 * b + 1]
        w[b, 0:HPC] = r0["w_out"]
        w[b, HPC:H] = r1["w_out"]
        out[b] = (r0["outT"] + r1["outT"]).T
    return out, w


def run(x, w_qkv, w_proj, trace=False):
    if trace:
        _install_ntff_hook()
    nc = _get_nc()
    in_maps = _prep_in_maps(x, w_qkv, w_proj)
    res = run_bass_kernel_spmd(nc, in_maps, core_ids=list(range(N_CORES)),
                               trace=trace)
    out, w = _assemble(res.results)
    return (out, w), res


def kernel(x, w_qkv, w_proj):
    (out, w), _ = run(np.asarray(x, dtype=np.float32),
                      np.asarray(w_qkv, dtype=np.float32),
                      np.asarray(w_proj, dtype=np.float32))
    return (out, w)


# revision 8
# speedup vs baseline: 1.1814x; 1.1814x over previous
"""Multi-head causal attention (B=4,T=2048,D=1024,H=16) on 8 TRN2 NeuronCores.

Sharding: data-parallel on B (4 batches x 2 core-groups), tensor-parallel on
heads (8 heads per core). Each core computes, for its (b, 8-head) slice:
  - w[8,2048,2048]: softmax attention weights (lower triangle written;
    upper triangle left unwritten -> returned as zeros by the pre-zeroed
    donated output buffer)
  - outT[1024,2048]: transposed partial projection output; host adds the two
    per-batch partials and transposes.

Self-contained: hardcodes all shapes; only imports the environment-provided
concourse/axon stack.
"""

import contextlib
import ctypes
import sys
import types

import numpy as np

import concourse.bass as bass
import concourse.mybir as mybir
import concourse.tile as tile
from concourse import bacc
from concourse.bass_utils import run_bass_kernel_spmd

F32 = mybir.dt.float32
F32R = mybir.dt.float32r
BF16 = mybir.dt.bfloat16
AF = mybir.ActivationFunctionType

B, T, D, H, DH = 4, 2048, 1024, 16, 64
HPC = 8          # heads per core
N_CORES = 8
SCALE = 1.0 / 8.0  # 1/sqrt(DH)
NEG = -1e30

_NC = None  # cached compiled Bass module


# ---------------------------------------------------------------- ntff hook
def _install_ntff_hook():
    """The agent image's antenv lacks axon_hooks; replicate the ctypes NTFF
    profile hook so run_bass_kernel_spmd(trace=True) works."""
    if "antenv.axon_hooks" in sys.modules:
        return
    so_path = "/opt/axon/libaxon_pjrt.so"
    try:
        lib = ctypes.CDLL(so_path)
        assert hasattr(lib, "axon_start_nrt_profile")
        lib.axon_start_nrt_profile.argtypes = [ctypes.POINTER(ctypes.c_int64), ctypes.c_size_t]
        lib.axon_start_nrt_profile.restype = ctypes.c_int64
        lib.axon_stop_nrt_profile.argtypes = [ctypes.c_char_p]
        lib.axon_stop_nrt_profile.restype = ctypes.c_int64

        @contextlib.contextmanager
        def _hook(output_dir, device_ids):
            import jax
            jax.devices()
            if device_ids:
                ids = (ctypes.c_int64 * len(device_ids))(*device_ids)
                rc = lib.axon_start_nrt_profile(ids, len(device_ids))
            else:
                rc = lib.axon_start_nrt_profile(None, 0)
            if rc != 0:
                raise RuntimeError(f"axon_start_nrt_profile rc={rc}")
            try:
                yield
            finally:
                n = lib.axon_stop_nrt_profile(str(output_dir).encode())
                print(f"ntff profile: {n} file(s) -> {output_dir}", file=sys.stderr)
    except Exception:
        _hook = None
    mod = types.ModuleType("antenv.axon_hooks")
    mod.get_axon_ntff_profile_hook = lambda: _hook
    mod.set_axon_ntff_profile_hook = lambda h: None
    sys.modules["antenv.axon_hooks"] = mod


# ---------------------------------------------------------------- builder
def _build():
    nc = bacc.Bacc("TRN2", target_bir_lowering=False, debug=False,
                   num_devices=N_CORES)
    xT_d = nc.dram_tensor("xT", [D, T], F32R, kind="ExternalInput").ap()
    wqk_d = nc.dram_tensor("wqk", [D, 2 * HPC * DH], F32R, kind="ExternalInput").ap()
    wv_d = nc.dram_tensor("wv", [D, HPC * DH], F32R, kind="ExternalInput").ap()
    wp_d = nc.dram_tensor("wp", [HPC * DH, D], F32R, kind="ExternalInput").ap()
    w_out = nc.dram_tensor("w_out", [HPC, T, T], F32, kind="ExternalOutput").ap()
    outT_d = nc.dram_tensor("outT", [D, T], F32, kind="ExternalOutput").ap()

    NQK = 2 * HPC * DH   # 1024 (q rows | k rows)
    NV = HPC * DH        # 512
    NT = T // 128        # 16 key tiles
    NCH = T // 512       # 4 query chunks

    with tile.TileContext(nc) as tc:
        with contextlib.ExitStack() as ctx:
            consts = ctx.enter_context(tc.tile_pool(name="consts", bufs=1))
            persist = ctx.enter_context(tc.tile_pool(name="persist", bufs=1))

            # constants: bf16 identity, transposed additive causal mask
            # maskN[x(query), y(key)] = 0 if x >= y else NEG
            maskN = consts.tile([128, 128], F32)
            nc.gpsimd.memset(maskN[:], 0.0)
            nc.gpsimd.affine_select(out=maskN[:], in_=maskN[:],
                                    compare_op=mybir.AluOpType.is_ge,
                                    fill=NEG, base=0, pattern=[[-1, 128]],
                                    channel_multiplier=1)
            # maskT[x(key), y(query)] = 0 if y >= x else NEG
            maskT = consts.tile([128, 128], F32)
            nc.gpsimd.memset(maskT[:], 0.0)
            nc.gpsimd.affine_select(out=maskT[:], in_=maskT[:],
                                    compare_op=mybir.AluOpType.is_ge,
                                    fill=NEG, base=0, pattern=[[1, 128]],
                                    channel_multiplier=-1)

            # persistent tensors
            qkT = [persist.tile([128, T], F32R, tag=f"qkT{g}", name=f"qkT{g}")
                   for g in range(8)]
            v_aug = [persist.tile([128, HPC, DH + 1], BF16, tag=f"vaug{k}",
                                    name=f"vaug{k}")
                     for k in range(NT)]

            # ---------------- phase A: QKV projections ----------------
            with contextlib.ExitStack() as pa:
                wpool = pa.enter_context(tc.tile_pool(name="wpool", bufs=1))
                xpool = pa.enter_context(tc.tile_pool(name="xpool", bufs=2))
                qk_ps = pa.enter_context(tc.tile_pool(name="qk_ps", bufs=2, space="PSUM"))
                v_ps = pa.enter_context(tc.tile_pool(name="v_ps", bufs=2, space="PSUM"))

                wqk_sb = wpool.tile([128, 8, NQK], F32R)
                nc.sync.dma_start(wqk_sb[:], wqk_d.rearrange("(c p) d -> p c d", p=128))
                wv_sb = wpool.tile([128, 8, NV], F32R)
                nc.sync.dma_start(wv_sb[:], wv_d.rearrange("(c p) d -> p c d", p=128))
                ones8 = wpool.tile([128, HPC], F32)
                nc.vector.memset(ones8[:], 1.0)

                for t4 in range(NCH):
                    xt = xpool.tile([128, 8, 512], F32R, tag="xt")
                    nc.sync.dma_start(
                        xt[:], xT_d[:, 512 * t4:512 * (t4 + 1)]
                        .rearrange("(c p) t -> p c t", p=128))
                    for dt in range(8):
                        P = qk_ps.tile([128, 512], F32, tag="qk")
                        for cc in range(8):
                            nc.tensor.matmul(P[:], wqk_sb[:, cc, 128 * dt:128 * (dt + 1)],
                                             xt[:, cc, :], start=(cc == 0), stop=(cc == 7))
                        nc.vector.tensor_copy(qkT[dt][:, 512 * t4:512 * (t4 + 1)], P[:])
                    for tt in range(4):
                        kt = 4 * t4 + tt
                        P = v_ps.tile([128, 512], F32, tag="v")
                        for cc in range(8):
                            nc.tensor.matmul(P[:], xt[:, cc, 128 * tt:128 * (tt + 1)],
                                             wv_sb[:, cc, :], start=(cc == 0), stop=(cc == 7))
                        nc.vector.tensor_copy(
                            v_aug[kt][:, :, 0:DH],
                            P[:].rearrange("p (h d) -> p h d", h=HPC))
                        nc.vector.tensor_copy(v_aug[kt][:, :, DH], ones8[:])

            # ---------------- phase B: attention + projection ----------------
            with contextlib.ExitStack() as pb:
                bpool = pb.enter_context(tc.tile_pool(name="bpool", bufs=1))
                ctxT = [bpool.tile([128, T], F32R, tag=f"ctxT{g}", name=f"ctxT{g}")
                        for g in range(4)]
                wp_sb = bpool.tile([128, 4, D], F32R)
                nc.sync.dma_start(wp_sb[:], wp_d.rearrange("(g p) e -> p g e", p=128))

                with contextlib.ExitStack() as pc:
                    et_pool = pc.enter_context(tc.tile_pool(name="et", bufs=6))
                    small = pc.enter_context(tc.tile_pool(name="small", bufs=3))
                    wstage = pc.enter_context(tc.tile_pool(name="wstage", bufs=2))
                    sT_ps = pc.enter_context(tc.tile_pool(name="sT_ps", bufs=2, space="PSUM"))
                    ctx_ps = pc.enter_context(tc.tile_pool(name="ctx_ps", bufs=2, space="PSUM"))
                    en_ps = pc.enter_context(tc.tile_pool(name="en_ps", bufs=2, space="PSUM"))

                    for hl in range(HPC):
                        g = hl // 2
                        base = (hl % 2) * 64
                        kTh = qkT[4 + g]
                        qTh = qkT[g]
                        for c in range(NCH):
                            q0 = 512 * c
                            jmax = 4 * c + 3
                            CP = ctx_ps.tile([DH + 1, 512], F32, tag="ctx")
                            for j in range(jmax + 1):
                                qs = max(q0, 128 * j)
                                N = 512 * (c + 1) - qs
                                d0 = qs - q0
                                S = sT_ps.tile([128, 512], F32, tag="sT")
                                nc.tensor.matmul(
                                    S[:, :N],
                                    kTh[base:base + 64, 128 * j:128 * (j + 1)],
                                    qTh[base:base + 64, qs:qs + N],
                                    start=True, stop=True, tile_position=(base, 0))
                                if 128 * j >= q0:  # diagonal block at cols 0:128
                                    nc.vector.tensor_add(S[:, 0:128], S[:, 0:128], maskT[:])
                                ET = et_pool.tile([128, 512], BF16, tag="et")
                                nc.scalar.activation(ET[:, :N], S[:, :N], AF.Exp, scale=SCALE)
                                nc.tensor.matmul(CP[:, d0:d0 + N],
                                                 v_aug[j][:, hl, :], ET[:, :N],
                                                 start=(j == 0), stop=(j == jmax))
                            # Z row -> rz (col + row forms) -> ctxT normalize
                            zrow = small.tile([1, 512], F32, tag="zrow")
                            nc.vector.tensor_copy(zrow[:], CP[DH:DH + 1, :])
                            zcol = small.tile([128, 4], F32, tag="zcol")
                            for f in range(4):
                                nc.sync.dma_start(zcol[:, f:f + 1],
                                                  zrow[:, 128 * f:128 * (f + 1)])
                            rzc = small.tile([128, 4], F32, tag="rzc")
                            nc.vector.reciprocal(rzc[:], zcol[:])
                            nlz = small.tile([128, 4], F32, tag="nlz")
                            nc.scalar.activation(nlz[:], rzc[:], AF.Ln)
                            rzrow = small.tile([1, 512], F32, tag="rzrow")
                            for f in range(4):
                                nc.sync.dma_start(rzrow[:, 128 * f:128 * (f + 1)],
                                                  rzc[:, f:f + 1])
                            bc = small.tile([64, 512], F32, tag="bc")
                            nc.gpsimd.partition_broadcast(bc[:], rzrow[:])
                            nc.vector.tensor_mul(ctxT[g][base:base + 64, q0:q0 + 512],
                                                 CP[0:DH, :], bc[:])
                            # natural-orientation w tiles: s = qT_i.T @ kT, then
                            # w = exp(s/8 - lnZ) straight out of ACT (pre-normalized)
                            for il in range(4):
                                i = 4 * c + il
                                nk = 128 * (i + 1)
                                W = wstage.tile([128, T], F32, tag="w")
                                for kb in range(0, nk, 1024):
                                    width = min(1024, nk - kb)
                                    EP = en_ps.tile([128, 1024], F32, tag="en")
                                    for s0 in range(0, width, 512):
                                        sw = min(512, width - s0)
                                        nc.tensor.matmul(
                                            EP[:, s0:s0 + sw],
                                            qTh[base:base + 64, 128 * i:128 * (i + 1)],
                                            kTh[base:base + 64, kb + s0:kb + s0 + sw],
                                            start=True, stop=True,
                                            tile_position=(base, 0))
                                    if kb + width == nk:  # diag block: last 128 cols
                                        nc.vector.tensor_add(
                                            EP[:, width - 128:width],
                                            EP[:, width - 128:width], maskN[:])
                                    nc.scalar.activation(
                                        W[:, kb:kb + width], EP[:, :width],
                                        AF.Exp, scale=SCALE, bias=nlz[:, il:il + 1])
                                nc.sync.dma_start(
                                    w_out[hl, 128 * i:128 * (i + 1), 0:nk],
                                    W[:, 0:nk])

                # ---------------- projection ----------------
                with contextlib.ExitStack() as pd:
                    opool = pd.enter_context(tc.tile_pool(name="opool", bufs=3))
                    o_ps = pd.enter_context(tc.tile_pool(name="o_ps", bufs=3, space="PSUM"))
                    for et8 in range(8):
                        for tn in range(NCH):
                            P = o_ps.tile([128, 512], F32, tag="o")
                            for g in range(4):
                                nc.tensor.matmul(P[:],
                                                 wp_sb[:, g, 128 * et8:128 * (et8 + 1)],
                                                 ctxT[g][:, 512 * tn:512 * (tn + 1)],
                                                 start=(g == 0), stop=(g == 3))
                            St = opool.tile([128, 512], F32, tag="st")
                            nc.vector.tensor_copy(St[:], P[:])
                            nc.sync.dma_start(
                                outT_d[128 * et8:128 * (et8 + 1), 512 * tn:512 * (tn + 1)],
                                St[:])
    nc.compile()
    return nc


def _get_nc():
    global _NC
    if _NC is None:
        _NC = _build()
    return _NC


def _prep_in_maps(x, w_qkv, w_proj):
    in_maps = []
    for core in range(N_CORES):
        b = core // 2
        hs = (core % 2) * HPC
        r0 = hs * DH
        wq = w_qkv[r0:r0 + 512]
        wk = w_qkv[D + r0:D + r0 + 512]
        wv = w_qkv[2 * D + r0:2 * D + r0 + 512]
        in_maps.append({
            "xT": np.ascontiguousarray(x[b].T),
            "wqk": np.ascontiguousarray(np.concatenate([wq, wk], axis=0).T),
            "wv": np.ascontiguousarray(wv.T),
            "wp": np.ascontiguousarray(w_proj[:, r0:r0 + 512].T),
        })
    return in_maps


def _assemble(results):
    w = np.empty((B, H, T, T), dtype=np.float32)
    out = np.empty((B, T, D), dtype=np.float32)
    for b in range(B):
        r0, r1 = results[2 * b], results[2 * b + 1]
        w[b, 0:HPC] = r0["w_out"]
        w[b, HPC:H] = r1["w_out"]
        out[b] = (r0["outT"] + r1["outT"]).T
    return out, w


def run(x, w_qkv, w_proj, trace=False):
    if trace:
        _install_ntff_hook()
    nc = _get_nc()
    in_maps = _prep_in_maps(x, w_qkv, w_proj)
    res = run_bass_kernel_spmd(nc, in_maps, core_ids=list(range(N_CORES)),
                               trace=trace)
    out, w = _assemble(res.results)
    return (out, w), res


def kernel(x, w_qkv, w_proj):
    (out, w), _ = run(np.asarray(x, dtype=np.float32),
                      np.asarray(w_qkv, dtype=np.float32),
                      np.asarray(w_proj, dtype=np.float32))
    return (out, w)


# revision 9
# speedup vs baseline: 1.2517x; 1.0595x over previous
"""Multi-head causal attention (B=4,T=2048,D=1024,H=16) on 8 TRN2 NeuronCores.

Sharding: data-parallel on B (4 batches x 2 core-groups), tensor-parallel on
heads (8 heads per core). Each core computes, for its (b, 8-head) slice:
  - w[8,2048,2048]: softmax attention weights (lower triangle written;
    upper triangle left unwritten -> returned as zeros by the pre-zeroed
    donated output buffer)
  - outT[1024,2048]: transposed partial projection output; host adds the two
    per-batch partials and transposes.

Self-contained: hardcodes all shapes; only imports the environment-provided
concourse/axon stack.
"""

import contextlib
import ctypes
import sys
import types

import numpy as np

import concourse.bass as bass
import concourse.mybir as mybir
import concourse.tile as tile
from concourse import bacc
from concourse.bass_utils import run_bass_kernel_spmd

F32 = mybir.dt.float32
F32R = mybir.dt.float32r
BF16 = mybir.dt.bfloat16
AF = mybir.ActivationFunctionType

B, T, D, H, DH = 4, 2048, 1024, 16, 64
HPC = 8          # heads per core
N_CORES = 8
SCALE = 1.0 / 8.0  # 1/sqrt(DH)
NEG = -1e30

_NC = None  # cached compiled Bass module


# ---------------------------------------------------------------- ntff hook
def _install_ntff_hook():
    """The agent image's antenv lacks axon_hooks; replicate the ctypes NTFF
    profile hook so run_bass_kernel_spmd(trace=True) works."""
    if "antenv.axon_hooks" in sys.modules:
        return
    so_path = "/opt/axon/libaxon_pjrt.so"
    try:
        lib = ctypes.CDLL(so_path)
        assert hasattr(lib, "axon_start_nrt_profile")
        lib.axon_start_nrt_profile.argtypes = [ctypes.POINTER(ctypes.c_int64), ctypes.c_size_t]
        lib.axon_start_nrt_profile.restype = ctypes.c_int64
        lib.axon_stop_nrt_profile.argtypes = [ctypes.c_char_p]
        lib.axon_stop_nrt_profile.restype = ctypes.c_int64

        @contextlib.contextmanager
        def _hook(output_dir, device_ids):
            import jax
            jax.devices()
            if device_ids:
                ids = (ctypes.c_int64 * len(device_ids))(*device_ids)
                rc = lib.axon_start_nrt_profile(ids, len(device_ids))
            else:
                rc = lib.axon_start_nrt_profile(None, 0)
            if rc != 0:
                raise RuntimeError(f"axon_start_nrt_profile rc={rc}")
            try:
                yield
            finally:
                n = lib.axon_stop_nrt_profile(str(output_dir).encode())
                print(f"ntff profile: {n} file(s) -> {output_dir}", file=sys.stderr)
    except Exception:
        _hook = None
    mod = types.ModuleType("antenv.axon_hooks")
    mod.get_axon_ntff_profile_hook = lambda: _hook
    mod.set_axon_ntff_profile_hook = lambda h: None
    sys.modules["antenv.axon_hooks"] = mod


# ---------------------------------------------------------------- builder
def _build():
    nc = bacc.Bacc("TRN2", target_bir_lowering=False, debug=False,
                   num_devices=N_CORES)
    xT_d = nc.dram_tensor("xT", [D, T], F32R, kind="ExternalInput").ap()
    wqk_d = nc.dram_tensor("wqk", [D, 2 * HPC * DH], F32R, kind="ExternalInput").ap()
    wv_d = nc.dram_tensor("wv", [D, HPC * DH], F32R, kind="ExternalInput").ap()
    wp_d = nc.dram_tensor("wp", [HPC * DH, D], F32R, kind="ExternalInput").ap()
    w_out = nc.dram_tensor("w_out", [HPC, T, T], F32, kind="ExternalOutput").ap()
    outT_d = nc.dram_tensor("outT", [D, T], F32, kind="ExternalOutput").ap()

    NQK = 2 * HPC * DH   # 1024 (q rows | k rows)
    NV = HPC * DH        # 512
    NT = T // 128        # 16 key tiles
    NCH = T // 512       # 4 query chunks

    with tile.TileContext(nc) as tc:
        with contextlib.ExitStack() as ctx:
            consts = ctx.enter_context(tc.tile_pool(name="consts", bufs=1))
            persist = ctx.enter_context(tc.tile_pool(name="persist", bufs=1))

            # constants: bf16 identity, transposed additive causal mask
            # maskN[x(query), y(key)] = 0 if x >= y else NEG
            maskN = consts.tile([128, 128], F32)
            nc.gpsimd.memset(maskN[:], 0.0)
            nc.gpsimd.affine_select(out=maskN[:], in_=maskN[:],
                                    compare_op=mybir.AluOpType.is_ge,
                                    fill=NEG, base=0, pattern=[[-1, 128]],
                                    channel_multiplier=1)
            # maskT[x(key), y(query)] = 0 if y >= x else NEG
            maskT = consts.tile([128, 128], F32)
            nc.gpsimd.memset(maskT[:], 0.0)
            nc.gpsimd.affine_select(out=maskT[:], in_=maskT[:],
                                    compare_op=mybir.AluOpType.is_ge,
                                    fill=NEG, base=0, pattern=[[1, 128]],
                                    channel_multiplier=-1)

            # persistent tensors
            qkT = [persist.tile([128, T], BF16, tag=f"qkT{g}", name=f"qkT{g}")
                   for g in range(8)]
            v_aug = [persist.tile([128, HPC, DH + 1], BF16, tag=f"vaug{k}",
                                    name=f"vaug{k}")
                     for k in range(NT)]

            # ---------------- phase A: QKV projections ----------------
            with contextlib.ExitStack() as pa:
                wpool = pa.enter_context(tc.tile_pool(name="wpool", bufs=1))
                xpool = pa.enter_context(tc.tile_pool(name="xpool", bufs=2))
                qk_ps = pa.enter_context(tc.tile_pool(name="qk_ps", bufs=2, space="PSUM"))
                v_ps = pa.enter_context(tc.tile_pool(name="v_ps", bufs=2, space="PSUM"))

                wqk_sb = wpool.tile([128, 8, NQK], F32R)
                nc.sync.dma_start(wqk_sb[:], wqk_d.rearrange("(c p) d -> p c d", p=128))
                wv_sb = wpool.tile([128, 8, NV], F32R)
                nc.sync.dma_start(wv_sb[:], wv_d.rearrange("(c p) d -> p c d", p=128))
                ones8 = wpool.tile([128, HPC], F32)
                nc.vector.memset(ones8[:], 1.0)

                for t4 in range(NCH):
                    xt = xpool.tile([128, 8, 512], F32R, tag="xt")
                    nc.sync.dma_start(
                        xt[:], xT_d[:, 512 * t4:512 * (t4 + 1)]
                        .rearrange("(c p) t -> p c t", p=128))
                    for dt in range(8):
                        P = qk_ps.tile([128, 512], F32, tag="qk")
                        for cc in range(8):
                            nc.tensor.matmul(P[:], wqk_sb[:, cc, 128 * dt:128 * (dt + 1)],
                                             xt[:, cc, :], start=(cc == 0), stop=(cc == 7))
                        nc.vector.tensor_copy(qkT[dt][:, 512 * t4:512 * (t4 + 1)], P[:])
                    for tt in range(4):
                        kt = 4 * t4 + tt
                        P = v_ps.tile([128, 512], F32, tag="v")
                        for cc in range(8):
                            nc.tensor.matmul(P[:], xt[:, cc, 128 * tt:128 * (tt + 1)],
                                             wv_sb[:, cc, :], start=(cc == 0), stop=(cc == 7))
                        nc.vector.tensor_copy(
                            v_aug[kt][:, :, 0:DH],
                            P[:].rearrange("p (h d) -> p h d", h=HPC))
                        nc.vector.tensor_copy(v_aug[kt][:, :, DH], ones8[:])

            # ---------------- phase B: attention + projection ----------------
            with contextlib.ExitStack() as pb:
                bpool = pb.enter_context(tc.tile_pool(name="bpool", bufs=1))
                ctxT = [bpool.tile([128, T], F32R, tag=f"ctxT{g}", name=f"ctxT{g}")
                        for g in range(4)]
                wp_sb = bpool.tile([128, 4, D], F32R)
                nc.sync.dma_start(wp_sb[:], wp_d.rearrange("(g p) e -> p g e", p=128))

                with contextlib.ExitStack() as pc:
                    et_pool = pc.enter_context(tc.tile_pool(name="et", bufs=6))
                    small = pc.enter_context(tc.tile_pool(name="small", bufs=3))
                    wstage = pc.enter_context(tc.tile_pool(name="wstage", bufs=2))
                    sT_ps = pc.enter_context(tc.tile_pool(name="sT_ps", bufs=2, space="PSUM"))
                    ctx_ps = pc.enter_context(tc.tile_pool(name="ctx_ps", bufs=2, space="PSUM"))
                    en_ps = pc.enter_context(tc.tile_pool(name="en_ps", bufs=2, space="PSUM"))

                    for hl in range(HPC):
                        g = hl // 2
                        base = (hl % 2) * 64
                        kTh = qkT[4 + g]
                        qTh = qkT[g]
                        for c in range(NCH):
                            q0 = 512 * c
                            jmax = 4 * c + 3
                            CP = ctx_ps.tile([DH + 1, 512], F32, tag="ctx")
                            for j in range(jmax + 1):
                                qs = max(q0, 128 * j)
                                N = 512 * (c + 1) - qs
                                d0 = qs - q0
                                S = sT_ps.tile([128, 512], F32, tag="sT")
                                nc.tensor.matmul(
                                    S[:, :N],
                                    kTh[base:base + 64, 128 * j:128 * (j + 1)],
                                    qTh[base:base + 64, qs:qs + N],
                                    start=True, stop=True, tile_position=(base, 0))
                                if 128 * j >= q0:  # diagonal block at cols 0:128
                                    nc.vector.tensor_add(S[:, 0:128], S[:, 0:128], maskT[:])
                                ET = et_pool.tile([128, 512], BF16, tag="et")
                                nc.scalar.activation(ET[:, :N], S[:, :N], AF.Exp, scale=SCALE)
                                nc.tensor.matmul(CP[:, d0:d0 + N],
                                                 v_aug[j][:, hl, :], ET[:, :N],
                                                 start=(j == 0), stop=(j == jmax))
                            # Z row -> rz (col + row forms) -> ctxT normalize
                            zrow = small.tile([1, 512], F32, tag="zrow")
                            nc.vector.tensor_copy(zrow[:], CP[DH:DH + 1, :])
                            zcol = small.tile([128, 4], F32, tag="zcol")
                            for f in range(4):
                                nc.sync.dma_start(zcol[:, f:f + 1],
                                                  zrow[:, 128 * f:128 * (f + 1)])
                            rzc = small.tile([128, 4], F32, tag="rzc")
                            nc.vector.reciprocal(rzc[:], zcol[:])
                            nlz = small.tile([128, 4], F32, tag="nlz")
                            nc.scalar.activation(nlz[:], rzc[:], AF.Ln)
                            rzrow = small.tile([1, 512], F32, tag="rzrow")
                            for f in range(4):
                                nc.sync.dma_start(rzrow[:, 128 * f:128 * (f + 1)],
                                                  rzc[:, f:f + 1])
                            bc = small.tile([64, 512], F32, tag="bc")
                            nc.gpsimd.partition_broadcast(bc[:], rzrow[:])
                            nc.vector.tensor_mul(ctxT[g][base:base + 64, q0:q0 + 512],
                                                 CP[0:DH, :], bc[:])
                            # natural-orientation w tiles: s = qT_i.T @ kT, then
                            # w = exp(s/8 - lnZ) straight out of ACT (pre-normalized)
                            for il in range(4):
                                i = 4 * c + il
                                nk = 128 * (i + 1)
                                W = wstage.tile([128, T], F32, tag="w")
                                for kb in range(0, nk, 1024):
                                    width = min(1024, nk - kb)
                                    EP = en_ps.tile([128, 1024], F32, tag="en")
                                    for s0 in range(0, width, 512):
                                        sw = min(512, width - s0)
                                        nc.tensor.matmul(
                                            EP[:, s0:s0 + sw],
                                            qTh[base:base + 64, 128 * i:128 * (i + 1)],
                                            kTh[base:base + 64, kb + s0:kb + s0 + sw],
                                            start=True, stop=True,
                                            tile_position=(base, 0))
                                    if kb + width == nk:  # diag block: last 128 cols
                                        nc.vector.tensor_add(
                                            EP[:, width - 128:width],
                                            EP[:, width - 128:width], maskN[:])
                                    nc.scalar.activation(
                                        W[:, kb:kb + width], EP[:, :width],
                                        AF.Exp, scale=SCALE, bias=nlz[:, il:il + 1])
                                nc.sync.dma_start(
                                    w_out[hl, 128 * i:128 * (i + 1), 0:nk],
                                    W[:, 0:nk])

                # ---------------- projection ----------------
                with contextlib.ExitStack() as pd:
                    opool = pd.enter_context(tc.tile_pool(name="opool", bufs=3))
                    o_ps = pd.enter_context(tc.tile_pool(name="o_ps", bufs=3, space="PSUM"))
                    for et8 in range(8):
                        for tn in range(NCH):
                            P = o_ps.tile([128, 512], F32, tag="o")
                            for g in range(4):
                                nc.tensor.matmul(P[:],
                                                 wp_sb[:, g, 128 * et8:128 * (et8 + 1)],
                                                 ctxT[g][:, 512 * tn:512 * (tn + 1)],
                                                 start=(g == 0), stop=(g == 3))
                            St = opool.tile([128, 512], F32, tag="st")
                            nc.vector.tensor_copy(St[:], P[:])
                            nc.sync.dma_start(
                                outT_d[128 * et8:128 * (et8 + 1), 512 * tn:512 * (tn + 1)],
                                St[:])
    nc.compile()
    return nc


def _get_nc():
    global _NC
    if _NC is None:
        _NC = _build()
    return _NC


def _prep_in_maps(x, w_qkv, w_proj):
    in_maps = []
    for core in range(N_CORES):
        b = core // 2
        hs = (core % 2) * HPC
        r0 = hs * DH
        wq = w_qkv[r0:r0 + 512]
        wk = w_qkv[D + r0:D + r0 + 512]
        wv = w_qkv[2 * D + r0:2 * D + r0 + 512]
        in_maps.append({
            "xT": np.ascontiguousarray(x[b].T),
            "wqk": np.ascontiguousarray(np.concatenate([wq, wk], axis=0).T),
            "wv": np.ascontiguousarray(wv.T),
            "wp": np.ascontiguousarray(w_proj[:, r0:r0 + 512].T),
        })
    return in_maps


def _assemble(results):
    w = np.empty((B, H, T, T), dtype=np.float32)
    out = np.empty((B, T, D), dtype=np.float32)
    for b in range(B):
        r0, r1 = results[2 * b], results[2 * b + 1]
        w[b, 0:HPC] = r0["w_out"]
        w[b, HPC:H] = r1["w_out"]
        out[b] = (r0["outT"] + r1["outT"]).T
    return out, w


def run(x, w_qkv, w_proj, trace=False):
    if trace:
        _install_ntff_hook()
    nc = _get_nc()
    in_maps = _prep_in_maps(x, w_qkv, w_proj)
    res = run_bass_kernel_spmd(nc, in_maps, core_ids=list(range(N_CORES)),
                               trace=trace)
    out, w = _assemble(res.results)
    return (out, w), res


def kernel(x, w_qkv, w_proj):
    (out, w), _ = run(np.asarray(x, dtype=np.float32),
                      np.asarray(w_qkv, dtype=np.float32),
                      np.asarray(w_proj, dtype=np.float32))
    return (out, w)
